# revision 22
# baseline (speedup 1.0000x reference)
"""Causal self-attention (B=4, S=2048, E=1024, H=16, hd=64) on 8 TRN2 NeuronCores.

Sharding: tensor-parallel over (batch, head-half). Core c handles batch c%4 and
heads [8*(c//4), 8*(c//4)+8) -- i.e. a 512-wide slice of the Wq/Wk/Wv columns
and of the Wo rows. Each core computes a partial [S, E] c_proj output; the host
sums the two half partials per batch and adds bo.

Single fused software pipeline over 4 groups (one 512-query chunk each):
  group g:
    per s-tile (4 tiles): x[128,E] DMA -> PE transpose (f32r) -> xT;
        v[128,512] = xT_tile^T-stationary matmul Wv (natural layout) + bv
        (DVE add of a matmul-broadcast bias row) -> v2 bf16 (+ ones col)
    prefetch next group's x tiles
    q/k projections for this 512-chunk only (weights streamed per (proj,pt))
    attention for query chunk g over key tiles 0..4g+3 (causal):
        S_T = kT^T-form matmul qT (scores transposed, bf16)
        P_T = exp(S_T)  (scalar engine; no max-subtraction: |scores| <~ 7)
        diagonal-strip causal mask via 0/1 trimask (DVE mul)
        yT_aug[65, :] += v_aug^T-form matmul P_T   (row 64 = softmax sums)
        normalize (copy sums row, K=1 matmul broadcast, fast reciprocal,
        DVE mul -> bf16 yT), emission deferred into the next head-pair's
        QK stream so the PE never waits on it
    c_proj for the 4 s-tiles of chunk g (bf16 x bf16) -> out DMA

Projection matmuls run as float32r (full PE rate at N>=256); attention and
c_proj run bf16.  x DMAs ride the SP hwdge queue, weight DMAs the Activation
queue, so both streams overlap.
"""

import numpy as np

import concourse.bass as bass
from concourse import bacc
import concourse.mybir as mybir
import concourse.tile as tile
from concourse.bass_utils import run_bass_kernel_spmd
from concourse.masks import make_identity

# Problem dims (hardcoded per contract)
B, S, E, H, HD = 4, 2048, 1024, 16, 64
NCORES = 8
EH = 512            # per-core slice of E (8 heads)
NHP = 4             # head pairs per core (2 heads share a 128-partition tile)
NPT = EH // 128     # 4 partition tiles of the per-core head slice
NKT = E // 128      # 8 contraction tiles over E
NST = S // 128      # 16 s-tiles
NIC = S // 512      # 4 query chunks
SCALE = 1.0 / np.sqrt(HD)

F32 = mybir.dt.float32
F32R = mybir.dt.float32r
BF16 = mybir.dt.bfloat16

_CACHED_NC = {}


def _mm(ap, mode):
    """Bitcast an AP to the matmul compute dtype (no-op if already typed)."""
    if mode == "fp32r" and ap.dtype != F32R:
        return ap.bitcast(F32R)
    return ap


def build_bass(mode="mixed"):
    """Build the single-core SPMD Bass program (same program on all 8 cores)."""
    nc = bacc.Bacc()
    x_h = nc.declare_dram_parameter("x", [S, E], F32, isOutput=False)
    wq_h = nc.declare_dram_parameter("wq", [E, EH], F32, isOutput=False)
    wk_h = nc.declare_dram_parameter("wk", [E, EH], F32, isOutput=False)
    wv_h = nc.declare_dram_parameter("wv", [E, EH], F32, isOutput=False)
    wo_h = nc.declare_dram_parameter("wo", [EH, E], F32, isOutput=False)
    bq_h = nc.declare_dram_parameter("bq", [EH], F32, isOutput=False)  # pre-scaled by 1/8
    bk_h = nc.declare_dram_parameter("bk", [EH], F32, isOutput=False)
    bv_h = nc.declare_dram_parameter("bv", [EH], F32, isOutput=False)
    out_h = nc.declare_dram_parameter("out", [S, E], F32, isOutput=True)

    with tile.TileContext(nc) as tc:
        _build_body(nc, tc, x_h, wq_h, wk_h, wv_h, wo_h, bq_h, bk_h, bv_h, out_h, mode)
    if not nc.is_finalized():
        nc.finalize()
    return nc


def _build_body(nc, tc, x_h, wq_h, wk_h, wv_h, wo_h, bq_h, bk_h, bv_h, out_h, mode):
    import contextlib

    MD = F32 if mode == "fp32" else F32R   # stationary (lhsT) tile dtype
    MV = BF16 if mode == "mixed" else MD   # moving (rhs) tile dtype

    Exp = mybir.ActivationFunctionType.Exp
    Copy = mybir.ActivationFunctionType.Copy
    Ident = mybir.ActivationFunctionType.Identity

    def mmd(ap):
        return ap if MD == F32 else ap.bitcast(F32R)

    with contextlib.ExitStack() as ctx:
        const = ctx.enter_context(tc.tile_pool(name="const", bufs=1))
        big = ctx.enter_context(tc.tile_pool(name="big", bufs=1))
        ps_grp = ctx.enter_context(tc.tile_pool(name="ps_grp", bufs=2, space="PSUM"))
        ps_y = ctx.enter_context(tc.tile_pool(name="ps_y", bufs=3, space="PSUM"))
        ps_bc = ctx.enter_context(tc.tile_pool(name="ps_bc", bufs=1, space="PSUM"))
        xs_pool = ctx.enter_context(tc.tile_pool(name="xs", bufs=4))
        w_pool = ctx.enter_context(tc.tile_pool(name="wstream", bufs=2))
        pt_pool = ctx.enter_context(tc.tile_pool(name="ptp", bufs=5))
        rc_pool = ctx.enter_context(tc.tile_pool(name="rcp", bufs=2))
        out_pool = ctx.enter_context(tc.tile_pool(name="outp", bufs=2))

        identity = const.tile([128, 128], F32, name="identity")
        make_identity(nc, identity[:])
        if MD == F32:
            identity_r = identity
        else:
            # f32r matmul operands must be *typed* f32r at their producer
            identity_r = const.tile([128, 128], F32R, name="identity_r")
            nc.vector.tensor_copy(identity_r[:], identity[:])
        # f32 ones staging tile (memset can't write bf16-typed v2 col)
        ones_f32 = const.tile([128, 128], F32, name="ones_f32")
        nc.gpsimd.memset(ones_f32[:], 1.0)
        # MD-typed ones used as K=1 lhsT for partition-broadcast matmuls
        ones_col = const.tile([128, 128], MD, name="ones_col")
        nc.vector.tensor_copy(ones_col[:], ones_f32[:])
        # upper-triangular (keep y >= p) multiplicative mask for the 128-wide
        # diagonal strip of each causal block
        mdt = F32 if MV == F32R else MV
        trimask = const.tile([128, 128], mdt, name="trimask")
        nc.gpsimd.memset(trimask[:], 1.0)
        nc.gpsimd.affine_select(
            out=trimask[:],
            in_=trimask[:],
            compare_op=mybir.AluOpType.is_ge,
            fill=0.0,
            base=0,
            pattern=[[1, 128]],
            channel_multiplier=-1,
        )
        # --- biases: single-row DMAs, PE transpose for the per-partition q/k
        # layout, K=1 matmul broadcast for the free-dim bv row ---
        brow = const.tile([2, EH], F32, name="brow")
        bvrow = const.tile([1, EH], MD, name="bvrow")
        nc.sync.dma_start(brow[0:1, :], bq_h[:].rearrange("(a m) -> a m", a=1))
        nc.sync.dma_start(brow[1:2, :], bk_h[:].rearrange("(a m) -> a m", a=1))
        nc.sync.dma_start(
            bvrow[0:1, :], bv_h[:].rearrange("(a m) -> a m", a=1).bitcast(MD)
        )
        bqk_sb = const.tile([128, NPT, 2], F32, name="bqk_sb")  # [p, pt, {q,k}]
        bv_bc = const.tile([128, EH], F32, name="bv_bc")
        ps_b2 = ps_grp.tile([128, 2, 512], F32, tag="grp", name="ps_b")
        ps_b = ps_b2[:, 0, :]
        for j in range(NPT):
            nc.tensor.transpose(
                ps_b[:, j * 2 : j * 2 + 2],
                brow[0:2, j * 128 : (j + 1) * 128],
                identity[0:2, 0:2],
            )
        nc.scalar.activation(
            bqk_sb[:], ps_b[:, 0 : 2 * NPT].rearrange("p (a b) -> p a b", a=NPT), Copy
        )
        ps_bv2 = ps_grp.tile([128, 2, 512], F32, tag="grp", name="ps_bv")
        nc.tensor.matmul(
            ps_bv2[:, 0, :],
            lhsT=ones_col[0:1, :],
            rhs=bvrow[0:1, :],
            start=True,
            stop=True,
        )
        nc.vector.tensor_copy(bv_bc[:], ps_bv2[:, 0, :])

        # Long-lived activation tensors
        xT = big.tile([128, NKT, S], MD, name="xT")   # [e-in-tile, kt, s]
        wv_sb = big.tile([128, NKT, EH], MD, name="wv_sb")
        qT = big.tile([128, NHP, S], MV, name="qT")   # [d(2 heads), hp, s]
        kT = big.tile([128, NHP, S], MV, name="kT")
        # v2: [s-in-tile, st, head (hp*2+hh), hd+1]; col 64 = ones (softmax sums)
        v2 = big.tile([128, NST, 2 * NHP, HD + 1], MV, name="v2")
        yT = big.tile([128, NPT, S], MV, name="yT")
        wo_sb = big.tile([128, NPT, E], MV, name="wo_sb")
        nc.vector.tensor_copy(
            v2[:, :, :, HD : HD + 1],
            ones_f32[:, 0 : NST * 2 * NHP].rearrange(
                "p (a b c) -> p a b c", a=NST, b=2 * NHP
            ),
        )
        # Wv streamed in two halves on the Activation hwdge queue so the first
        # v matmuls don't wait on the full 2MB
        wv_r = wv_h[:].rearrange("(ko p) m -> p ko m", p=128)
        nc.scalar.dma_start(wv_sb[:, 0:4, :], wv_r[:, 0:4, :].bitcast(MD))
        nc.scalar.dma_start(wv_sb[:, 4:8, :], wv_r[:, 4:8, :].bitcast(MD))

        # x tile prefetch (SP hwdge queue)
        x_tiles = {}

        def prefetch_x(g):
            for st in range(4 * g, 4 * g + 4):
                x_t = xs_pool.tile([128, E], MD, tag="xs")
                nc.sync.dma_start(
                    x_t[:], x_h[:][st * 128 : (st + 1) * 128, :].bitcast(MD)
                )
                x_tiles[st] = x_t

        prefetch_x(0)

        w_r = {
            "q": wq_h[:].rearrange("(ko p) m -> p ko m", p=128),
            "k": wk_h[:].rearrange("(ko p) m -> p ko m", p=128),
        }
        wo_r = wo_h[:].rearrange("(ko p) n -> p ko n", p=128)

        for g in range(NIC):
            _sc = nc.enter_named_scope(f"grp{g}", False)[0]
            # ---- s-tile transposes + v projection ----
            for st in range(4 * g, 4 * g + 4):
                x_t = x_tiles.pop(st)
                for kg in range(2):
                    ps2t = ps_grp.tile([128, 2, 512], F32, tag="grp", name="ps2t")
                    ps = ps2t[:, 0, :]
                    for j in range(4):
                        kt = kg * 4 + j
                        nc.tensor.transpose(
                            mmd(ps[:, j * 128 : (j + 1) * 128]),
                            mmd(x_t[:, kt * 128 : (kt + 1) * 128]),
                            identity_r[:],
                        )
                    dst = xT[:, kg * 4 : (kg + 1) * 4, st * 128 : (st + 1) * 128]
                    src = ps[:].rearrange("p (a b) -> p a b", a=4)
                    if kg == 0:
                        nc.scalar.activation(dst, src, Copy)
                    else:
                        nc.vector.tensor_copy(dst, src)
                accv2 = ps_grp.tile([128, 2, 512], F32, tag="grp", name="accv")
                accv = accv2[:, 0, :]
                for kt in range(NKT):
                    nc.tensor.matmul(
                        accv[:],
                        lhsT=_mm(xT[:, kt, st * 128 : (st + 1) * 128], mode),
                        rhs=_mm(wv_sb[:, kt, :], mode),
                        start=(kt == 0),
                        stop=(kt == NKT - 1),
                    )
                nc.vector.tensor_add(
                    v2[:, st, :, 0:HD],
                    accv[:].rearrange("p (a b) -> p a b", a=2 * NHP),
                    bv_bc[:].rearrange("p (a b) -> p a b", a=2 * NHP),
                )
            if g == 0:
                # Wo load (+ bf16 convert) staged through the x-tile pool --
                # must run before prefetch_x(1) claims these buffers, else the
                # buffer hand-off deadlocks against c_proj(0)'s need for wo_sb
                if MV == BF16:
                    for j in range(NPT):
                        wchunk = xs_pool.tile([128, E], MD, tag="xs")
                        nc.scalar.dma_start(wchunk[:], wo_r[:, j, :].bitcast(MD))
                        nc.vector.tensor_copy(wo_sb[:, j, :], wchunk[:].bitcast(F32))
                else:
                    nc.scalar.dma_start(wo_sb[:], wo_r.bitcast(MD))
            if g + 1 < NIC:
                prefetch_x(g + 1)

            # ---- q/k projection for this 512-chunk ----
            for pname in ("q", "k"):
                outT = qT if pname == "q" else kT
                bcol = 0 if pname == "q" else 1
                p_scale = SCALE if pname == "q" else 1.0
                for pt in range(NPT):
                    wt = w_pool.tile([128, NKT, 128], MD, tag="w")
                    nc.scalar.dma_start(
                        wt[:], w_r[pname][:, :, pt * 128 : (pt + 1) * 128].bitcast(MD)
                    )
                    acc2 = ps_grp.tile([128, 2, 512], F32, tag="grp", name="acc2")
                    acc = acc2[:, 0, :]
                    for kt in range(NKT):
                        nc.tensor.matmul(
                            acc[:],
                            lhsT=_mm(wt[:, kt, :], mode),
                            rhs=_mm(xT[:, kt, g * 512 : (g + 1) * 512], mode),
                            start=(kt == 0),
                            stop=(kt == NKT - 1),
                        )
                    nc.scalar.activation(
                        outT[:, pt, g * 512 : (g + 1) * 512],
                        acc[:],
                        Ident,
                        bias=bqk_sb[:, pt, bcol : bcol + 1],
                        scale=p_scale,
                    )

            # ---- attention for query chunk ic=g ----
            ic = g
            pending_norm = []

            def flush_norm():
                while pending_norm:
                    pending_norm.pop(0)()

            for hp in range(NHP):
                njt = 4 * ic + 4
                psy = [
                    ps_y.tile([128, 512], F32, tag="y", name=f"psy{i}")
                    for i in range(2)
                ]
                stage = []  # pending (jt, ps_s, pt_t) awaiting exp+PV

                def flush(jt_p, ps_s, pt_t, njt=njt, ic=ic, hp=hp, psy=psy):
                    r = jt_p - 4 * ic
                    w0 = 128 * r if r >= 0 else 0
                    W = 512 - w0
                    # one wide exp covers both heads' score halves
                    nc.scalar.activation(
                        pt_t[:, :, w0:512],
                        ps_s[:, :, 0:W],
                        Exp,
                    )
                    if r >= 0:
                        # only the first 128 cols of the window are partially
                        # masked (j <= 127 < i elsewhere)
                        for hh in range(2):
                            nc.vector.tensor_mul(
                                pt_t[:, hh, w0 : w0 + 128],
                                pt_t[:, hh, w0 : w0 + 128],
                                trimask[:],
                            )
                    for hh in range(2):
                        nc.tensor.matmul(
                            psy[hh][0:HD + 1, w0:512],
                            lhsT=_mm(v2[:, jt_p, hp * 2 + hh, :], mode),
                            rhs=_mm(pt_t[:, hh, w0:512], mode),
                            start=(jt_p == 0),
                            stop=(jt_p == njt - 1),
                        )

                for jt in range(njt):
                    r = jt - 4 * ic
                    w0 = 128 * r if r >= 0 else 0
                    W = 512 - w0
                    ps_s = ps_grp.tile([128, 2, 512], F32, tag="grp")
                    for hh in range(2):
                        base = hh * 64
                        nc.tensor.matmul(
                            ps_s[:, hh, 0:W],
                            lhsT=_mm(
                                kT[base : base + 64, hp, jt * 128 : (jt + 1) * 128],
                                mode,
                            ),
                            rhs=_mm(
                                qT[base : base + 64, hp, ic * 512 + w0 : (ic + 1) * 512],
                                mode,
                            ),
                            start=True,
                            stop=True,
                        )
                    if jt == 0:
                        # previous head-pair's normalize drops in here, after
                        # this head-pair's first QK so the PE never idles on it
                        flush_norm()
                    pt_t = pt_pool.tile([128, 2, 512], MV, tag="pt")
                    stage.append((jt, ps_s, pt_t))
                    if len(stage) > 1:
                        flush(*stage.pop(0))
                while stage:
                    flush(*stage.pop(0))

                def norm(hp=hp, ic=ic, psy=psy):
                    # normalize: yT = psy[0:64] * (1 / psy[64])
                    for hh in range(2):
                        srow = rc_pool.tile([128, 512], MD, tag="srow")
                        nc.vector.tensor_copy(srow[64:65, :], psy[hh][64:65, :])
                        bc_t = ps_bc.tile([128, 512], F32, tag="bc")
                        nc.tensor.matmul(
                            bc_t[0:64, :],
                            lhsT=ones_col[64:65, 0:64],
                            rhs=srow[64:65, :],
                            start=True,
                            stop=True,
                        )
                        rrow = rc_pool.tile([64, 512], F32, tag="rrow")
                        nc.vector.reciprocal_approx_fast(rrow[:, :], bc_t[0:64, :])
                        nc.vector.tensor_mul(
                            yT[hh * 64 : hh * 64 + 64, hp, ic * 512 : (ic + 1) * 512],
                            psy[hh][0:64, :],
                            rrow[:, :],
                        )

                pending_norm.append(norm)

            flush_norm()

            # ---- c_proj for the 4 s-tiles of this query chunk ----
            for st in range(4 * ic, 4 * ic + 4):
                for ec in range(2):
                    acc2 = ps_grp.tile([128, 2, 512], F32, tag="grp", name="acc2c")
                    acc = acc2[:, 0, :]
                    for ptd in range(NPT):
                        nc.tensor.matmul(
                            acc[:],
                            lhsT=_mm(yT[:, ptd, st * 128 : (st + 1) * 128], mode),
                            rhs=_mm(wo_sb[:, ptd, ec * 512 : (ec + 1) * 512], mode),
                            start=(ptd == 0),
                            stop=(ptd == NPT - 1),
                        )
                    ot = out_pool.tile([128, 512], F32, tag="ot")
                    if ec == 0:
                        nc.scalar.activation(ot[:], acc[:], Copy)
                    else:
                        nc.vector.tensor_copy(ot[:], acc[:])
                    nc.sync.dma_start(
                        out_h[:][st * 128 : (st + 1) * 128, ec * 512 : (ec + 1) * 512],
                        ot[:],
                    )
            nc.leave_named_scope(f"grp{g}", _sc, False)


def _get_nc(mode="mixed"):
    if mode not in _CACHED_NC:
        _CACHED_NC[mode] = build_bass(mode)
    return _CACHED_NC[mode]


def make_in_maps(x, Wq, bq, Wk, bk, Wv, bv, Wo, bo):
    in_maps = []
    for c in range(NCORES):
        b = c % B
        half = c // B
        sl = slice(half * EH, (half + 1) * EH)
        in_maps.append(
            {
                "x": np.ascontiguousarray(x[b]),
                "wq": np.ascontiguousarray(Wq[:, sl]),
                "wk": np.ascontiguousarray(Wk[:, sl]),
                "wv": np.ascontiguousarray(Wv[:, sl]),
                "wo": np.ascontiguousarray(Wo[sl, :]),
                "bq": np.ascontiguousarray(bq[sl]) * np.float32(SCALE),
                "bk": np.ascontiguousarray(bk[sl]),
                "bv": np.ascontiguousarray(bv[sl]),
            }
        )
    return in_maps


def assemble(results, bo):
    out = np.empty((B, S, E), dtype=np.float32)
    for b in range(B):
        out[b] = results[b]["out"] + results[b + B]["out"] + bo[None, :]
    return out


def kernel(x, Wq, bq, Wk, bk, Wv, bv, Wo, bo, _trace=False, _mode="mixed"):
    x = np.asarray(x, dtype=np.float32)
    Wq = np.asarray(Wq, dtype=np.float32)
    bq = np.asarray(bq, dtype=np.float32)
    Wk = np.asarray(Wk, dtype=np.float32)
    bk = np.asarray(bk, dtype=np.float32)
    Wv = np.asarray(Wv, dtype=np.float32)
    bv = np.asarray(bv, dtype=np.float32)
    Wo = np.asarray(Wo, dtype=np.float32)
    bo = np.asarray(bo, dtype=np.float32)

    nc = _get_nc(_mode)
    in_maps = make_in_maps(x, Wq, bq, Wk, bk, Wv, bv, Wo, bo)
    res = run_bass_kernel_spmd(nc, in_maps, list(range(NCORES)), trace=_trace)
    out = assemble(res.results, bo)
    if _trace:
        return out, res
    return out


# revision 25
# speedup vs baseline: 1.0382x; 1.0382x over previous
"""Causal self-attention (B=4, S=2048, E=1024, H=16, hd=64) on 8 TRN2 NeuronCores.

Sharding: tensor-parallel over (batch, head-half). Core c handles batch c%4 and
heads [8*(c//4), 8*(c//4)+8) -- i.e. a 512-wide slice of the Wq/Wk/Wv columns
and of the Wo rows. Each core computes a partial [S, E] c_proj output; the host
sums the two half partials per batch and adds bo.

Single fused software pipeline over 4 groups (one 512-query chunk each):
  group g:
    per s-tile (4 tiles): x[128,E] DMA -> PE transpose (f32r) -> xT;
        v[128,512] = xT_tile^T-stationary matmul Wv (natural layout) + bv
        (DVE add of a matmul-broadcast bias row) -> v2 bf16 (+ ones col)
    prefetch next group's x tiles
    q/k projections for this 512-chunk only (weights streamed per (proj,pt))
    attention for query chunk g over key tiles 0..4g+3 (causal):
        S_T = kT^T-form matmul qT (scores transposed, bf16)
        P_T = exp(S_T)  (scalar engine; no max-subtraction: |scores| <~ 7)
        diagonal-strip causal mask via 0/1 trimask (DVE mul)
        yT_aug[65, :] += v_aug^T-form matmul P_T   (row 64 = softmax sums)
        normalize (copy sums row, K=1 matmul broadcast, fast reciprocal,
        DVE mul -> bf16 yT), emission deferred into the next head-pair's
        QK stream so the PE never waits on it
    c_proj for the 4 s-tiles of chunk g (bf16 x bf16) -> out DMA

Projection matmuls run as float32r (full PE rate at N>=256); attention and
c_proj run bf16.  x DMAs ride the SP hwdge queue, weight DMAs the Activation
queue, so both streams overlap.
"""

import numpy as np

import concourse.bass as bass
from concourse import bacc
import concourse.mybir as mybir
import concourse.tile as tile
from concourse.bass_utils import run_bass_kernel_spmd
from concourse.masks import make_identity

# Problem dims (hardcoded per contract)
B, S, E, H, HD = 4, 2048, 1024, 16, 64
NCORES = 8
EH = 512            # per-core slice of E (8 heads)
NHP = 4             # head pairs per core (2 heads share a 128-partition tile)
NPT = EH // 128     # 4 partition tiles of the per-core head slice
NKT = E // 128      # 8 contraction tiles over E
NST = S // 128      # 16 s-tiles
NIC = S // 512      # 4 query chunks
SCALE = 1.0 / np.sqrt(HD)

F32 = mybir.dt.float32
F32R = mybir.dt.float32r
BF16 = mybir.dt.bfloat16

_CACHED_NC = {}


def _mm(ap, mode):
    """Bitcast an AP to the matmul compute dtype (no-op if already typed)."""
    if mode == "fp32r" and ap.dtype != F32R:
        return ap.bitcast(F32R)
    return ap


def build_bass(mode="mixed"):
    """Build the single-core SPMD Bass program (same program on all 8 cores)."""
    nc = bacc.Bacc()
    x_h = nc.declare_dram_parameter("x", [S, E], F32, isOutput=False)
    wq_h = nc.declare_dram_parameter("wq", [E, EH], F32, isOutput=False)
    wk_h = nc.declare_dram_parameter("wk", [E, EH], F32, isOutput=False)
    wv_h = nc.declare_dram_parameter("wv", [E, EH], F32, isOutput=False)
    wo_h = nc.declare_dram_parameter("wo", [EH, E], F32, isOutput=False)
    bq_h = nc.declare_dram_parameter("bq", [EH], F32, isOutput=False)  # pre-scaled by 1/8
    bk_h = nc.declare_dram_parameter("bk", [EH], F32, isOutput=False)
    bv_h = nc.declare_dram_parameter("bv", [EH], F32, isOutput=False)
    out_h = nc.declare_dram_parameter("out", [S, E], F32, isOutput=True)

    with tile.TileContext(nc) as tc:
        _build_body(nc, tc, x_h, wq_h, wk_h, wv_h, wo_h, bq_h, bk_h, bv_h, out_h, mode)
    if not nc.is_finalized():
        nc.finalize()
    return nc


def _build_body(nc, tc, x_h, wq_h, wk_h, wv_h, wo_h, bq_h, bk_h, bv_h, out_h, mode):
    import contextlib

    MD = F32 if mode == "fp32" else F32R   # stationary (lhsT) tile dtype
    MV = BF16 if mode == "mixed" else MD   # moving (rhs) tile dtype

    Exp = mybir.ActivationFunctionType.Exp
    Copy = mybir.ActivationFunctionType.Copy
    Ident = mybir.ActivationFunctionType.Identity

    def mmd(ap):
        return ap if MD == F32 else ap.bitcast(F32R)

    with contextlib.ExitStack() as ctx:
        const = ctx.enter_context(tc.tile_pool(name="const", bufs=1))
        big = ctx.enter_context(tc.tile_pool(name="big", bufs=1))
        ps_grp = ctx.enter_context(tc.tile_pool(name="ps_grp", bufs=3, space="PSUM"))
        ps_y = ctx.enter_context(tc.tile_pool(name="ps_y", bufs=2, space="PSUM"))
        xs_pool = ctx.enter_context(tc.tile_pool(name="xs", bufs=3))
        w_pool = ctx.enter_context(tc.tile_pool(name="wstream", bufs=3))
        pt_pool = ctx.enter_context(tc.tile_pool(name="ptp", bufs=4))
        rc_pool = ctx.enter_context(tc.tile_pool(name="rcp", bufs=2))
        out_pool = ctx.enter_context(tc.tile_pool(name="outp", bufs=2))

        identity = const.tile([128, 128], F32, name="identity")
        make_identity(nc, identity[:])
        if MD == F32:
            identity_r = identity
        else:
            # f32r matmul operands must be *typed* f32r at their producer
            identity_r = const.tile([128, 128], F32R, name="identity_r")
            nc.vector.tensor_copy(identity_r[:], identity[:])
        # f32 ones staging tile (memset can't write bf16-typed v2 col)
        ones_f32 = const.tile([128, 128], F32, name="ones_f32")
        nc.gpsimd.memset(ones_f32[:], 1.0)
        # MD-typed ones used as K=1 lhsT for partition-broadcast matmuls
        ones_col = const.tile([128, 128], MD, name="ones_col")
        nc.vector.tensor_copy(ones_col[:], ones_f32[:])
        # upper-triangular (keep y >= p) multiplicative mask for the 128-wide
        # diagonal strip of each causal block
        mdt = F32 if MV == F32R else MV
        trimask = const.tile([128, 128], mdt, name="trimask")
        nc.gpsimd.memset(trimask[:], 1.0)
        nc.gpsimd.affine_select(
            out=trimask[:],
            in_=trimask[:],
            compare_op=mybir.AluOpType.is_ge,
            fill=0.0,
            base=0,
            pattern=[[1, 128]],
            channel_multiplier=-1,
        )
        # --- biases: single-row DMAs, PE transpose for the per-partition q/k
        # layout, K=1 matmul broadcast for the free-dim bv row ---
        brow = const.tile([2, EH], F32, name="brow")
        bvrow = const.tile([1, EH], MD, name="bvrow")
        nc.sync.dma_start(brow[0:1, :], bq_h[:].rearrange("(a m) -> a m", a=1))
        nc.sync.dma_start(brow[1:2, :], bk_h[:].rearrange("(a m) -> a m", a=1))
        nc.sync.dma_start(
            bvrow[0:1, :], bv_h[:].rearrange("(a m) -> a m", a=1).bitcast(MD)
        )
        bqk_sb = const.tile([128, NPT, 2], F32, name="bqk_sb")  # [p, pt, {q,k}]
        bv_bc = const.tile([128, EH], F32, name="bv_bc")
        ps_b2 = ps_grp.tile([128, 2, 512], F32, tag="grp", name="ps_b")
        ps_b = ps_b2[:, 0, :]
        for j in range(NPT):
            nc.tensor.transpose(
                ps_b[:, j * 2 : j * 2 + 2],
                brow[0:2, j * 128 : (j + 1) * 128],
                identity[0:2, 0:2],
            )
        nc.scalar.activation(
            bqk_sb[:], ps_b[:, 0 : 2 * NPT].rearrange("p (a b) -> p a b", a=NPT), Copy
        )
        ps_bv2 = ps_grp.tile([128, 2, 512], F32, tag="grp", name="ps_bv")
        nc.tensor.matmul(
            ps_bv2[:, 0, :],
            lhsT=ones_col[0:1, :],
            rhs=bvrow[0:1, :],
            start=True,
            stop=True,
        )
        nc.vector.tensor_copy(bv_bc[:], ps_bv2[:, 0, :])

        # Long-lived activation tensors
        xT = big.tile([128, NKT, S], MD, name="xT")   # [e-in-tile, kt, s]
        wv_sb = big.tile([128, NKT, EH], MD, name="wv_sb")
        qT = big.tile([128, NHP, S], MV, name="qT")   # [d(2 heads), hp, s]
        kT = big.tile([128, NHP, S], MV, name="kT")
        # v2: [s-in-tile, st, head (hp*2+hh), hd+1]; col 64 = ones (softmax sums)
        v2 = big.tile([128, NST, 2 * NHP, HD + 1], MV, name="v2")
        yT = big.tile([128, NPT, S], MV, name="yT")
        wo_sb = big.tile([128, NPT, E], MV, name="wo_sb")
        nc.vector.tensor_copy(
            v2[:, :, :, HD : HD + 1],
            ones_f32[:, 0 : NST * 2 * NHP].rearrange(
                "p (a b c) -> p a b c", a=NST, b=2 * NHP
            ),
        )
        # Wv streamed in two halves on the Activation hwdge queue so the first
        # v matmuls don't wait on the full 2MB
        wv_r = wv_h[:].rearrange("(ko p) m -> p ko m", p=128)
        nc.scalar.dma_start(wv_sb[:, 0:4, :], wv_r[:, 0:4, :].bitcast(MD))
        nc.scalar.dma_start(wv_sb[:, 4:8, :], wv_r[:, 4:8, :].bitcast(MD))

        # x tile prefetch (SP hwdge queue)
        x_tiles = {}

        def prefetch_x(g):
            for st in range(4 * g, 4 * g + 4):
                x_t = xs_pool.tile([128, E], MD, tag="xs")
                nc.sync.dma_start(
                    x_t[:], x_h[:][st * 128 : (st + 1) * 128, :].bitcast(MD)
                )
                x_tiles[st] = x_t

        prefetch_x(0)

        w_r = {
            "q": wq_h[:].rearrange("(ko p) m -> p ko m", p=128),
            "k": wk_h[:].rearrange("(ko p) m -> p ko m", p=128),
        }
        wo_r = wo_h[:].rearrange("(ko p) n -> p ko n", p=128)

        for g in range(NIC):
            _sc = nc.enter_named_scope(f"grp{g}", False)[0]
            # ---- s-tile transposes + v projection ----
            for st in range(4 * g, 4 * g + 4):
                x_t = x_tiles.pop(st)
                for kg in range(2):
                    ps2t = ps_grp.tile([128, 2, 512], F32, tag="grp", name="ps2t")
                    ps = ps2t[:, 0, :]
                    for j in range(4):
                        kt = kg * 4 + j
                        nc.tensor.transpose(
                            mmd(ps[:, j * 128 : (j + 1) * 128]),
                            mmd(x_t[:, kt * 128 : (kt + 1) * 128]),
                            identity_r[:],
                        )
                    dst = xT[:, kg * 4 : (kg + 1) * 4, st * 128 : (st + 1) * 128]
                    src = ps[:].rearrange("p (a b) -> p a b", a=4)
                    if kg == 0:
                        nc.scalar.activation(dst, src, Copy)
                    else:
                        nc.vector.tensor_copy(dst, src)
                accv2 = ps_grp.tile([128, 2, 512], F32, tag="grp", name="accv")
                accv = accv2[:, 0, :]
                for kt in range(NKT):
                    nc.tensor.matmul(
                        accv[:],
                        lhsT=_mm(xT[:, kt, st * 128 : (st + 1) * 128], mode),
                        rhs=_mm(wv_sb[:, kt, :], mode),
                        start=(kt == 0),
                        stop=(kt == NKT - 1),
                    )
                nc.vector.tensor_add(
                    v2[:, st, :, 0:HD],
                    accv[:].rearrange("p (a b) -> p a b", a=2 * NHP),
                    bv_bc[:].rearrange("p (a b) -> p a b", a=2 * NHP),
                )
            if g == 0:
                # Wo load (+ bf16 convert) staged through the x-tile pool --
                # must run before prefetch_x(1) claims these buffers, else the
                # buffer hand-off deadlocks against c_proj(0)'s need for wo_sb
                if MV == BF16:
                    for j in range(NPT):
                        wchunk = xs_pool.tile([128, E], MD, tag="xs")
                        nc.scalar.dma_start(wchunk[:], wo_r[:, j, :].bitcast(MD))
                        nc.vector.tensor_copy(wo_sb[:, j, :], wchunk[:].bitcast(F32))
                else:
                    nc.scalar.dma_start(wo_sb[:], wo_r.bitcast(MD))
            if g + 1 < NIC:
                prefetch_x(g + 1)

            # ---- q/k projection for this 512-chunk ----
            for pname in ("q", "k"):
                outT = qT if pname == "q" else kT
                bcol = 0 if pname == "q" else 1
                p_scale = SCALE if pname == "q" else 1.0
                for pt in range(NPT):
                    wt = w_pool.tile([128, NKT, 128], MD, tag="w")
                    nc.sync.dma_start(
                        wt[:], w_r[pname][:, :, pt * 128 : (pt + 1) * 128].bitcast(MD)
                    )
                    acc2 = ps_grp.tile([128, 2, 512], F32, tag="grp", name="acc2")
                    acc = acc2[:, 0, :]
                    for kt in range(NKT):
                        nc.tensor.matmul(
                            acc[:],
                            lhsT=_mm(wt[:, kt, :], mode),
                            rhs=_mm(xT[:, kt, g * 512 : (g + 1) * 512], mode),
                            start=(kt == 0),
                            stop=(kt == NKT - 1),
                        )
                    nc.scalar.activation(
                        outT[:, pt, g * 512 : (g + 1) * 512],
                        acc[:],
                        Ident,
                        bias=bqk_sb[:, pt, bcol : bcol + 1],
                        scale=p_scale,
                    )

            # ---- attention for query chunk ic=g ----
            ic = g
            pending_norm = []

            def flush_norm():
                while pending_norm:
                    pending_norm.pop(0)()

            for hp in range(NHP):
                njt = 4 * ic + 4
                psy = [
                    ps_y.tile([128, 512], F32, tag="y", name=f"psy{i}")
                    for i in range(2)
                ]
                stage = []  # pending (jt, ps_s, pt_t) awaiting exp+PV

                def flush(jt_p, ps_s, pt_t, njt=njt, ic=ic, hp=hp, psy=psy):
                    r = jt_p - 4 * ic
                    w0 = 128 * r if r >= 0 else 0
                    W = 512 - w0
                    # one wide exp covers both heads' score halves
                    nc.scalar.activation(
                        pt_t[:, :, w0:512],
                        ps_s[:, :, 0:W],
                        Exp,
                    )
                    if r >= 0:
                        # only the first 128 cols of the window are partially
                        # masked (j <= 127 < i elsewhere)
                        for hh in range(2):
                            nc.vector.tensor_mul(
                                pt_t[:, hh, w0 : w0 + 128],
                                pt_t[:, hh, w0 : w0 + 128],
                                trimask[:],
                            )
                    for hh in range(2):
                        nc.tensor.matmul(
                            psy[hh][0:HD + 1, w0:512],
                            lhsT=_mm(v2[:, jt_p, hp * 2 + hh, :], mode),
                            rhs=_mm(pt_t[:, hh, w0:512], mode),
                            start=(jt_p == 0),
                            stop=(jt_p == njt - 1),
                        )

                for jt in range(njt):
                    r = jt - 4 * ic
                    w0 = 128 * r if r >= 0 else 0
                    W = 512 - w0
                    ps_s = ps_grp.tile([128, 2, 512], F32, tag="grp")
                    for hh in range(2):
                        base = hh * 64
                        nc.tensor.matmul(
                            ps_s[:, hh, 0:W],
                            lhsT=_mm(
                                kT[base : base + 64, hp, jt * 128 : (jt + 1) * 128],
                                mode,
                            ),
                            rhs=_mm(
                                qT[base : base + 64, hp, ic * 512 + w0 : (ic + 1) * 512],
                                mode,
                            ),
                            start=True,
                            stop=True,
                        )
                    if jt == 0:
                        # previous head-pair's normalize drops in here, after
                        # this head-pair's first QK so the PE never idles on it
                        flush_norm()
                    pt_t = pt_pool.tile([128, 2, 512], MV, tag="pt")
                    stage.append((jt, ps_s, pt_t))
                    if len(stage) > 2:
                        flush(*stage.pop(0))
                while stage:
                    flush(*stage.pop(0))

                def norm(hp=hp, ic=ic, psy=psy):
                    # normalize: yT = psy[0:64] * (1 / psy[64])
                    for hh in range(2):
                        # one scratch tile: sums row lives at partition 64
                        # (f32r-typed for the matmul), reciprocal lands in
                        # rows 0:64 viewed as plain f32
                        srow = rc_pool.tile([128, 512], MD, tag="srow")
                        nc.vector.tensor_copy(srow[64:65, :], psy[hh][64:65, :])
                        bc_t2 = ps_grp.tile([128, 2, 512], F32, tag="grp", name="bc_t")
                        bc_t = bc_t2[:, 0, :]
                        nc.tensor.matmul(
                            bc_t[0:64, :],
                            lhsT=ones_col[64:65, 0:64],
                            rhs=srow[64:65, :],
                            start=True,
                            stop=True,
                        )
                        rrow = rc_pool.tile([64, 512], F32, tag="rrow")
                        nc.vector.reciprocal_approx_fast(rrow[:, :], bc_t[0:64, :])
                        nc.vector.tensor_mul(
                            yT[hh * 64 : hh * 64 + 64, hp, ic * 512 : (ic + 1) * 512],
                            psy[hh][0:64, :],
                            rrow[:, :],
                        )

                pending_norm.append(norm)

            flush_norm()

            # ---- c_proj for the 4 s-tiles of this query chunk ----
            for st in range(4 * ic, 4 * ic + 4):
                for ec in range(2):
                    acc2 = ps_grp.tile([128, 2, 512], F32, tag="grp", name="acc2c")
                    acc = acc2[:, 0, :]
                    for ptd in range(NPT):
                        nc.tensor.matmul(
                            acc[:],
                            lhsT=_mm(yT[:, ptd, st * 128 : (st + 1) * 128], mode),
                            rhs=_mm(wo_sb[:, ptd, ec * 512 : (ec + 1) * 512], mode),
                            start=(ptd == 0),
                            stop=(ptd == NPT - 1),
                        )
                    ot = out_pool.tile([128, 512], F32, tag="ot")
                    if ec == 0:
                        nc.scalar.activation(ot[:], acc[:], Copy)
                    else:
                        nc.vector.tensor_copy(ot[:], acc[:])
                    nc.sync.dma_start(
                        out_h[:][st * 128 : (st + 1) * 128, ec * 512 : (ec + 1) * 512],
                        ot[:],
                    )
            nc.leave_named_scope(f"grp{g}", _sc, False)


def _get_nc(mode="mixed"):
    if mode not in _CACHED_NC:
        _CACHED_NC[mode] = build_bass(mode)
    return _CACHED_NC[mode]


def make_in_maps(x, Wq, bq, Wk, bk, Wv, bv, Wo, bo):
    in_maps = []
    for c in range(NCORES):
        b = c % B
        half = c // B
        sl = slice(half * EH, (half + 1) * EH)
        in_maps.append(
            {
                "x": np.ascontiguousarray(x[b]),
                "wq": np.ascontiguousarray(Wq[:, sl]),
                "wk": np.ascontiguousarray(Wk[:, sl]),
                "wv": np.ascontiguousarray(Wv[:, sl]),
                "wo": np.ascontiguousarray(Wo[sl, :]),
                "bq": np.ascontiguousarray(bq[sl]) * np.float32(SCALE),
                "bk": np.ascontiguousarray(bk[sl]),
                "bv": np.ascontiguousarray(bv[sl]),
            }
        )
    return in_maps


def assemble(results, bo):
    out = np.empty((B, S, E), dtype=np.float32)
    for b in range(B):
        out[b] = results[b]["out"] + results[b + B]["out"] + bo[None, :]
    return out


def kernel(x, Wq, bq, Wk, bk, Wv, bv, Wo, bo, _trace=False, _mode="mixed"):
    x = np.asarray(x, dtype=np.float32)
    Wq = np.asarray(Wq, dtype=np.float32)
    bq = np.asarray(bq, dtype=np.float32)
    Wk = np.asarray(Wk, dtype=np.float32)
    bk = np.asarray(bk, dtype=np.float32)
    Wv = np.asarray(Wv, dtype=np.float32)
    bv = np.asarray(bv, dtype=np.float32)
    Wo = np.asarray(Wo, dtype=np.float32)
    bo = np.asarray(bo, dtype=np.float32)

    nc = _get_nc(_mode)
    in_maps = make_in_maps(x, Wq, bq, Wk, bk, Wv, bv, Wo, bo)
    res = run_bass_kernel_spmd(nc, in_maps, list(range(NCORES)), trace=_trace)
    out = assemble(res.results, bo)
    if _trace:
        return out, res
    return out


# revision 26
# speedup vs baseline: 1.1323x; 1.0907x over previous
"""Causal self-attention (B=4, S=2048, E=1024, H=16, hd=64) on 8 TRN2 NeuronCores.

Sharding: tensor-parallel over (batch, head-half). Core c handles batch c%4 and
heads [8*(c//4), 8*(c//4)+8) -- i.e. a 512-wide slice of the Wq/Wk/Wv columns
and of the Wo rows. Each core computes a partial [S, E] c_proj output; the host
sums the two half partials per batch and adds bo (the "all-reduce" is a
host-side add of 8 x 32MB partials, which is trivial next to kernel time).

Per-core kernel structure (one batch, 8 heads):
  per s-tile: x[128,E] DMA -> PE-transpose (f32r) -> xT[E-tiles, 128]
              v[128, 512] = xT_tile^T-stationary matmul Wv (natural layout,
              no re-transpose) + bv (DVE add of a partition-broadcast row),
              written to v2 as bf16 with a ones column (softmax sums)
  qT = (Wq_sl^T @ xT) * (1/8) + bq_sl*(1/8)   [512, S]   (scale folded in)
  kT = Wk_sl^T @ xT + bk_sl                   [512, S]
  per head, per 512-wide query chunk ic, over key tiles jt (causal):
     S_T[j, i] = kT_h[:, jt]^T-form matmul qT_h[:, ic]   (scores transposed)
     P_T = exp(S_T)            (no max-subtraction needed: |scores| <~ 7)
     causal mask on diagonal tiles via a precomputed 0/1 strip mask (DVE mul)
     yT_aug[65, ic] += v_aug_h[jt]^T-form matmul P_T     (row 64 = softmax sums)
  yT = yT_aug[0:64] * recip(yT_aug[64])  (DVE reciprocal on the psum row,
       gpsimd partition_broadcast, DVE mul -> bf16)
  out_partial = yT^T-form matmul Wo_sl   [S, E]  (bf16 x bf16; bo added on host)

Matmuls run as float32r for the projections (full PE rate at N>=256) and bf16
for attention + c_proj.
"""

import numpy as np

import concourse.bass as bass
from concourse import bacc
import concourse.mybir as mybir
import concourse.tile as tile
from concourse.bass_utils import run_bass_kernel_spmd
from concourse.masks import make_identity

# Problem dims (hardcoded per contract)
B, S, E, H, HD = 4, 2048, 1024, 16, 64
NCORES = 8
EH = 512            # per-core slice of E (8 heads)
NHP = 4             # head pairs per core (2 heads share a 128-partition tile)
NPT = EH // 128     # 4 partition tiles of the per-core head slice
NKT = E // 128      # 8 contraction tiles over E
NST = S // 128      # 16 s-tiles
NIC = S // 512      # 4 query chunks
SCALE = 1.0 / np.sqrt(HD)

F32 = mybir.dt.float32
F32R = mybir.dt.float32r
BF16 = mybir.dt.bfloat16

_CACHED_NC = {}


def _mm(ap, mode):
    """Bitcast an AP to the matmul compute dtype (no-op if already typed)."""
    if mode == "fp32r" and ap.dtype != F32R:
        return ap.bitcast(F32R)
    return ap


def build_bass(mode="mixed"):
    """Build the single-core SPMD Bass program (same program on all 8 cores)."""
    nc = bacc.Bacc()
    x_h = nc.declare_dram_parameter("x", [S, E], F32, isOutput=False)
    wq_h = nc.declare_dram_parameter("wq", [E, EH], F32, isOutput=False)
    wk_h = nc.declare_dram_parameter("wk", [E, EH], F32, isOutput=False)
    wv_h = nc.declare_dram_parameter("wv", [E, EH], F32, isOutput=False)
    wo_h = nc.declare_dram_parameter("wo", [EH, E], F32, isOutput=False)
    bq_h = nc.declare_dram_parameter("bq", [EH], F32, isOutput=False)  # pre-scaled by 1/8
    bk_h = nc.declare_dram_parameter("bk", [EH], F32, isOutput=False)
    bv_h = nc.declare_dram_parameter("bv", [EH], F32, isOutput=False)
    out_h = nc.declare_dram_parameter("out", [S, E], F32, isOutput=True)

    with tile.TileContext(nc) as tc:
        _build_body(nc, tc, x_h, wq_h, wk_h, wv_h, wo_h, bq_h, bk_h, bv_h, out_h, mode)
    if not nc.is_finalized():
        nc.finalize()
    return nc


def _build_body(nc, tc, x_h, wq_h, wk_h, wv_h, wo_h, bq_h, bk_h, bv_h, out_h, mode):
    import contextlib

    MD = F32 if mode == "fp32" else F32R   # stationary (lhsT) tile dtype
    MV = BF16 if mode == "mixed" else MD   # moving (rhs) tile dtype

    Exp = mybir.ActivationFunctionType.Exp
    Copy = mybir.ActivationFunctionType.Copy
    Ident = mybir.ActivationFunctionType.Identity

    def mmd(ap):
        # transpose/matmul inputs in the stationary dtype (f32r unless fp32 mode)
        return ap if MD == F32 else ap.bitcast(F32R)

    with contextlib.ExitStack() as ctx:
        const = ctx.enter_context(tc.tile_pool(name="const", bufs=1))
        big = ctx.enter_context(tc.tile_pool(name="big", bufs=1))
        ps_grp = ctx.enter_context(tc.tile_pool(name="ps_grp", bufs=3, space="PSUM"))
        ps_y = ctx.enter_context(tc.tile_pool(name="ps_y", bufs=2, space="PSUM"))

        identity = const.tile([128, 128], F32, name="identity")
        make_identity(nc, identity[:])
        if MD == F32:
            identity_r = identity
        else:
            # f32r matmul operands must be *typed* f32r at their producer;
            # a DVE copy into an f32r tile satisfies the BIR verifier
            identity_r = const.tile([128, 128], F32R, name="identity_r")
            nc.vector.tensor_copy(identity_r[:], identity[:])
        # f32 ones staging tile (memset can't write bf16-typed v2 col)
        ones_f32 = const.tile([128, 128], F32, name="ones_f32")
        nc.gpsimd.memset(ones_f32[:], 1.0)
        # MD-typed ones used as K=1 lhsT for partition-broadcast matmuls
        ones_col = const.tile([128, 128], MD, name="ones_col")
        nc.vector.tensor_copy(ones_col[:], ones_f32[:])
        # upper-triangular (keep y >= p) multiplicative mask for the 128-wide
        # diagonal strip of each causal block
        mdt = F32 if MV == F32R else MV
        trimask = const.tile([128, 128], mdt, name="trimask")
        nc.gpsimd.memset(trimask[:], 1.0)
        nc.gpsimd.affine_select(
            out=trimask[:],
            in_=trimask[:],
            compare_op=mybir.AluOpType.is_ge,
            fill=0.0,
            base=0,
            pattern=[[1, 128]],
            channel_multiplier=-1,
        )
        # --- biases: single-row DMAs (1 descriptor each), then a PE transpose
        # for the per-partition q/k layout and a partition-broadcast for bv ---
        brow = const.tile([2, EH], F32, name="brow")
        bvrow = const.tile([1, EH], MD, name="bvrow")
        nc.sync.dma_start(brow[0:1, :], bq_h[:].rearrange("(a m) -> a m", a=1))
        nc.sync.dma_start(brow[1:2, :], bk_h[:].rearrange("(a m) -> a m", a=1))
        nc.sync.dma_start(
            bvrow[0:1, :], bv_h[:].rearrange("(a m) -> a m", a=1).bitcast(MD)
        )
        bqk_sb = const.tile([128, NPT, 2], F32, name="bqk_sb")  # [p, pt, {q,k}]
        bv_bc = const.tile([128, EH], F32, name="bv_bc")
        ps_b2 = ps_grp.tile([128, 2, 512], F32, tag="grp", name="ps_b")
        ps_b = ps_b2[:, 0, :]
        for j in range(NPT):
            nc.tensor.transpose(
                ps_b[:, j * 2 : j * 2 + 2],
                brow[0:2, j * 128 : (j + 1) * 128],
                identity[0:2, 0:2],
            )
        nc.scalar.activation(
            bqk_sb[:], ps_b[:, 0 : 2 * NPT].rearrange("p (a b) -> p a b", a=NPT), Copy
        )
        # broadcast bv across 128 partitions via a K=1 matmul (ones lhsT)
        ps_bv2 = ps_grp.tile([128, 2, 512], F32, tag="grp", name="ps_bv")
        nc.tensor.matmul(
            ps_bv2[:, 0, :],
            lhsT=ones_col[0:1, :],
            rhs=bvrow[0:1, :],
            start=True,
            stop=True,
        )
        nc.vector.tensor_copy(bv_bc[:], ps_bv2[:, 0, :])

        # Long-lived activation tensors
        qT = big.tile([128, NHP, S], MV, name="qT")   # [d(2 heads), hp, s]
        kT = big.tile([128, NHP, S], MV, name="kT")
        # v2: [s-in-tile, st, head (hp*2+hh), hd+1]; col 64 = ones (softmax sums)
        v2 = big.tile([128, NST, 2 * NHP, HD + 1], MV, name="v2")
        nc.vector.tensor_copy(
            v2[:, :, :, HD : HD + 1],
            ones_f32[:, 0 : NST * 2 * NHP].rearrange(
                "p (a b c) -> p a b c", a=NST, b=2 * NHP
            ),
        )

        # ---- Phase A: x transpose + v projection (natural layout), per s-tile ----
        with tc.tile_pool(name="xphase", bufs=1) as xph, \
             tc.tile_pool(name="xs", bufs=4) as xs_pool, \
             tc.tile_pool(name="wstream", bufs=2) as w_pool:
            xT = xph.tile([128, NKT, S], MD, name="xT")  # [e-in-tile, kt, s]
            wv_sb = xph.tile([128, NKT, EH], MD, name="wv_sb")
            # weights go through the Activation hwdge queue so they stream in
            # parallel with the x tiles on the SP queue (two halves so the
            # first v matmuls don't wait on the full 2MB)
            wv_r = wv_h[:].rearrange("(ko p) m -> p ko m", p=128)
            nc.scalar.dma_start(wv_sb[:, 0:4, :], wv_r[:, 0:4, :].bitcast(MD))
            nc.scalar.dma_start(wv_sb[:, 4:8, :], wv_r[:, 4:8, :].bitcast(MD))

            _sc = nc.enter_named_scope("xpose_v", False)[0]
            for st in range(NST):
                x_t = xs_pool.tile([128, E], MD, tag="xs")
                nc.sync.dma_start(
                    x_t[:], x_h[:][st * 128 : (st + 1) * 128, :].bitcast(MD)
                )
                for kg in range(2):
                    ps2t = ps_grp.tile([128, 2, 512], F32, tag="grp", name="ps2t")
                    ps = ps2t[:, 0, :]
                    for j in range(4):
                        kt = kg * 4 + j
                        nc.tensor.transpose(
                            mmd(ps[:, j * 128 : (j + 1) * 128]),
                            mmd(x_t[:, kt * 128 : (kt + 1) * 128]),
                            identity_r[:],
                        )
                    # psum group -> xT[:, kg*4:(kg+1)*4, st*128:(st+1)*128]
                    dst = xT[:, kg * 4 : (kg + 1) * 4, st * 128 : (st + 1) * 128]
                    src = ps[:].rearrange("p (a b) -> p a b", a=4)
                    if kg == 0:
                        nc.scalar.activation(dst, src, Copy)
                    else:
                        nc.vector.tensor_copy(dst, src)
                # v natural-layout projection for this s-tile
                accv2 = ps_grp.tile([128, 2, 512], F32, tag="grp", name="accv")
                accv = accv2[:, 0, :]
                for kt in range(NKT):
                    nc.tensor.matmul(
                        accv[:],
                        lhsT=_mm(xT[:, kt, st * 128 : (st + 1) * 128], mode),
                        rhs=_mm(wv_sb[:, kt, :], mode),
                        start=(kt == 0),
                        stop=(kt == NKT - 1),
                    )
                nc.vector.tensor_add(
                    v2[:, st, :, 0:HD],
                    accv[:].rearrange("p (a b) -> p a b", a=2 * NHP),
                    bv_bc[:].rearrange("p (a b) -> p a b", a=2 * NHP),
                )

            nc.leave_named_scope("xpose_v", _sc, False)
            _sc = nc.enter_named_scope("proj", False)[0]
            # q/k projections: loop proj -> pt -> sc; weights streamed per (proj, pt)
            projs = [
                ("q", wq_h, 0, SCALE, qT),
                ("k", wk_h, 1, 1.0, kT),
            ]
            for pname, w_h, bcol, p_scale, outT in projs:
                w_r = w_h[:].rearrange("(ko p) m -> p ko m", p=128)
                for pt in range(NPT):
                    wt = w_pool.tile([128, NKT, 128], MD, tag="w")
                    nc.scalar.dma_start(wt[:], w_r[:, :, pt * 128 : (pt + 1) * 128].bitcast(MD))
                    for sc in range(NIC):
                        acc2 = ps_grp.tile([128, 2, 512], F32, tag="grp", name="acc2")
                        acc = acc2[:, 0, :]
                        for kt in range(NKT):
                            nc.tensor.matmul(
                                acc[:],
                                lhsT=_mm(wt[:, kt, :], mode),
                                rhs=_mm(xT[:, kt, sc * 512 : (sc + 1) * 512], mode),
                                start=(kt == 0),
                                stop=(kt == NKT - 1),
                            )
                        nc.scalar.activation(
                            outT[:, pt, sc * 512 : (sc + 1) * 512],
                            acc[:],
                            Ident,
                            bias=bqk_sb[:, pt, bcol : bcol + 1],
                            scale=p_scale,
                        )

            nc.leave_named_scope("proj", _sc, False)

        # ---------------- Phase C+D: attention + output projection ----------------
        with tc.tile_pool(name="att", bufs=1) as att_pool, \
             tc.tile_pool(name="ptp", bufs=6) as pt_pool, \
             tc.tile_pool(name="rcp", bufs=4) as rc_pool, \
             tc.tile_pool(name="wop", bufs=1) as wo_pool, \
             tc.tile_pool(name="outp", bufs=3) as out_pool:
            yT = att_pool.tile([128, NPT, S], MV, name="yT")
            wo_sb = wo_pool.tile([128, NPT, E], MV, name="wo_sb")
            if MV == BF16:
                wo_f32 = wo_pool.tile([128, NPT, E], F32, name="wo_f32")
                nc.scalar.dma_start(
                    wo_f32[:], wo_h[:].rearrange("(ko p) n -> p ko n", p=128)
                )
                nc.vector.tensor_copy(wo_sb[:], wo_f32[:])
            else:
                nc.scalar.dma_start(
                    wo_sb[:], wo_h[:].rearrange("(ko p) n -> p ko n", p=128).bitcast(MD)
                )

            for ic in range(NIC):
                _sc = nc.enter_named_scope(f"attn{ic}", False)[0]
                pending_norm = []

                def flush_norm(pending_norm=pending_norm):
                    while pending_norm:
                        pending_norm.pop(0)()

                for hp in range(NHP):
                    njt = 4 * ic + 4
                    psy = [
                        ps_y.tile([128, 512], F32, tag="y", name=f"psy{i}")
                        for i in range(2)
                    ]
                    stage = []  # pending (jt, ps_s, pt_t) awaiting exp+PV

                    def flush(jt_p, ps_s, pt_t, njt=njt, ic=ic, hp=hp, psy=psy):
                        r = jt_p - 4 * ic
                        w0 = 128 * r if r >= 0 else 0
                        W = 512 - w0
                        # one wide exp covers both heads' score halves
                        nc.scalar.activation(
                            pt_t[:, :, w0:512],
                            ps_s[:, :, 0:W],
                            Exp,
                        )
                        if r >= 0:
                            # only the first 128 cols of the window are
                            # partially masked (j <= 127 < i elsewhere)
                            for hh in range(2):
                                nc.vector.tensor_mul(
                                    pt_t[:, hh, w0 : w0 + 128],
                                    pt_t[:, hh, w0 : w0 + 128],
                                    trimask[:],
                                )
                        for hh in range(2):
                            nc.tensor.matmul(
                                psy[hh][0:HD + 1, w0:512],
                                lhsT=_mm(v2[:, jt_p, hp * 2 + hh, :], mode),
                                rhs=_mm(pt_t[:, hh, w0:512], mode),
                                start=(jt_p == 0),
                                stop=(jt_p == njt - 1),
                            )

                    for jt in range(njt):
                        r = jt - 4 * ic
                        w0 = 128 * r if r >= 0 else 0
                        W = 512 - w0
                        ps_s = ps_grp.tile([128, 2, 512], F32, tag="grp")
                        for hh in range(2):
                            base = hh * 64
                            nc.tensor.matmul(
                                ps_s[:, hh, 0:W],
                                lhsT=_mm(
                                    kT[base : base + 64, hp, jt * 128 : (jt + 1) * 128],
                                    mode,
                                ),
                                rhs=_mm(
                                    qT[base : base + 64, hp, ic * 512 + w0 : (ic + 1) * 512],
                                    mode,
                                ),
                                start=True,
                                stop=True,
                            )
                        if jt == 0:
                            # previous head-pair's normalize drops in here so
                            # the PE never idles waiting for it
                            flush_norm()
                        pt_t = pt_pool.tile([128, 2, 512], MV, tag="pt")
                        stage.append((jt, ps_s, pt_t))
                        if len(stage) > 2:
                            flush(*stage.pop(0))
                    while stage:
                        flush(*stage.pop(0))

                    def norm(hp=hp, ic=ic, psy=psy):
                      # normalize: yT = psy[0:64] * (1 / psy[64])
                      for hh in range(2):
                        srow = rc_pool.tile([128, 512], MD, tag="srow")
                        # sums row to SBUF (lane 64 -> lane 64)
                        nc.vector.tensor_copy(srow[64:65, :], psy[hh][64:65, :])
                        # K=1 matmul broadcasts the sums row across 64 psum
                        # partitions; fast DVE reciprocal lands it in SBUF
                        bc_ps2 = ps_grp.tile([128, 2, 512], F32, tag="grp", name="bc_ps2")
                        bc_ps = bc_ps2[:, 0, :]
                        nc.tensor.matmul(
                            bc_ps[0:64, :],
                            lhsT=ones_col[64:65, 0:64],
                            rhs=srow[64:65, :],
                            start=True,
                            stop=True,
                        )
                        rrow = rc_pool.tile([64, 512], F32, tag="rrow")
                        nc.vector.reciprocal_approx_fast(
                            rrow[:, :], bc_ps[0:64, :]
                        )
                        nc.vector.tensor_mul(
                            yT[hh * 64 : hh * 64 + 64, hp, ic * 512 : (ic + 1) * 512],
                            psy[hh][0:64, :],
                            rrow[:, :],
                        )

                    pending_norm.append(norm)

                flush_norm()

                # c_proj for the 4 s-tiles of this query chunk
                for st in range(4 * ic, 4 * ic + 4):
                    ot = out_pool.tile([128, E], F32, tag="ot")
                    for ec in range(2):
                        acc2 = ps_grp.tile([128, 2, 512], F32, tag="grp", name="acc2c")
                        acc = acc2[:, 0, :]
                        for ptd in range(NPT):
                            nc.tensor.matmul(
                                acc[:],
                                lhsT=_mm(yT[:, ptd, st * 128 : (st + 1) * 128], mode),
                                rhs=_mm(wo_sb[:, ptd, ec * 512 : (ec + 1) * 512], mode),
                                start=(ptd == 0),
                                stop=(ptd == NPT - 1),
                            )
                        if ec == 0:
                            nc.scalar.activation(ot[:, 0:512], acc[:], Copy)
                        else:
                            nc.vector.tensor_copy(ot[:, 512:1024], acc[:])
                    nc.sync.dma_start(out_h[:][st * 128 : (st + 1) * 128, :], ot[:])
                nc.leave_named_scope(f"attn{ic}", _sc, False)


def _get_nc(mode="mixed"):
    if mode not in _CACHED_NC:
        _CACHED_NC[mode] = build_bass(mode)
    return _CACHED_NC[mode]


def make_in_maps(x, Wq, bq, Wk, bk, Wv, bv, Wo, bo):
    in_maps = []
    for c in range(NCORES):
        b = c % B
        half = c // B
        sl = slice(half * EH, (half + 1) * EH)
        in_maps.append(
            {
                "x": np.ascontiguousarray(x[b]),
                "wq": np.ascontiguousarray(Wq[:, sl]),
                "wk": np.ascontiguousarray(Wk[:, sl]),
                "wv": np.ascontiguousarray(Wv[:, sl]),
                "wo": np.ascontiguousarray(Wo[sl, :]),
                "bq": np.ascontiguousarray(bq[sl]) * np.float32(SCALE),
                "bk": np.ascontiguousarray(bk[sl]),
                "bv": np.ascontiguousarray(bv[sl]),
            }
        )
    return in_maps


def assemble(results, bo):
    out = np.empty((B, S, E), dtype=np.float32)
    for b in range(B):
        out[b] = results[b]["out"] + results[b + B]["out"] + bo[None, :]
    return out


def kernel(x, Wq, bq, Wk, bk, Wv, bv, Wo, bo, _trace=False, _mode="mixed"):
    x = np.asarray(x, dtype=np.float32)
    Wq = np.asarray(Wq, dtype=np.float32)
    bq = np.asarray(bq, dtype=np.float32)
    Wk = np.asarray(Wk, dtype=np.float32)
    bk = np.asarray(bk, dtype=np.float32)
    Wv = np.asarray(Wv, dtype=np.float32)
    bv = np.asarray(bv, dtype=np.float32)
    Wo = np.asarray(Wo, dtype=np.float32)
    bo = np.asarray(bo, dtype=np.float32)

    nc = _get_nc(_mode)
    in_maps = make_in_maps(x, Wq, bq, Wk, bk, Wv, bv, Wo, bo)
    res = run_bass_kernel_spmd(nc, in_maps, list(range(NCORES)), trace=_trace)
    out = assemble(res.results, bo)
    if _trace:
        return out, res
    return out
